# revision 7
# baseline (speedup 1.0000x reference)
"""Raw-Bacc (no TileContext) CenterLoss kernel.

loss = mean_b ||x_b - centers[labels_b]||^2  (+ tiny clip-floor constant)

Per core (128 batch rows), mode "fused_bf16":
  SP queue:  labels[0:64] spray, then x rows 0:64   (ring-FIFO: labels first)
  ACT queue: labels[64:128] spray, then x rows 64:128
  DVE:       prefill d = -x (bf16) while labels complete
  Pool:      indirect DMA gather with CCE compute_op=add: d += centers[labels]
             (the drain itself produces diff = c - x, cast to bf16)
  DVE:       sq = d*d with f32 row-accumulate -> s1
  PE:        ones-matmul reduces the 128 partition values to one scalar
  DVE:       PSUM -> SBUF copy;  SP: DMA scalar out
Host sums the 8 per-core partials (the all-reduce) and divides by B.

mode "diff_bf16" keeps the gather a plain cast (no CCE op) and computes
diff = c - x on the DVE instead.  mode "fused_f32" is the fusion without
the bf16 cast.
"""

import numpy as np

_BATCH = 1024
_FEAT = 512
_NCLASSES = 10000
_NCORES = 8
_ROWS = _BATCH // _NCORES  # 128
_P = 128
_H = _ROWS // 2  # 64

MODE = "diff_bf16"

_state = {}


def _build_nc_raw(mode=MODE):
    import concourse.bass as bass
    import concourse.mybir as mybir
    from concourse import bacc

    f32 = mybir.dt.float32
    bf16 = mybir.dt.bfloat16
    i32 = mybir.dt.int32
    fused = mode in ("fused_bf16", "fused_f32")
    gdt = f32 if mode == "fused_f32" else bf16

    nc = bacc.Bacc("TRN2", target_bir_lowering=False, debug=False)
    x_d = nc.dram_tensor("x", [_ROWS, _FEAT], f32, kind="ExternalInput").ap()
    labels_d = nc.dram_tensor("labels", [_ROWS, 1], i32, kind="ExternalInput").ap()
    centers_d = nc.dram_tensor(
        "centers", [_NCLASSES, _FEAT], f32, kind="ExternalInput"
    ).ap()
    out_d = nc.dram_tensor("out", [1, 1], f32, kind="ExternalOutput").ap()

    from contextlib import ExitStack

    with ExitStack() as _es:
        labels_t = _es.enter_context(nc.sbuf_tensor("labels_t", [_ROWS, 1], i32))
        x_t = _es.enter_context(nc.sbuf_tensor("x_t", [_P, _FEAT], f32))
        d_t = _es.enter_context(nc.sbuf_tensor("d_t", [_P, _FEAT], gdt))
        sq_t = _es.enter_context(nc.sbuf_tensor("sq_t", [_P, _FEAT], gdt))
        s1_t = _es.enter_context(nc.sbuf_tensor("s1_t", [_P, 1], f32))
        ones_t = _es.enter_context(nc.sbuf_tensor("ones_t", [_P, 1], f32))
        res_t = _es.enter_context(nc.sbuf_tensor("res_t", [1, 1], f32))
        acc_t = _es.enter_context(nc.psum_tensor("acc_t", [1, 1], f32))
        if not fused:
            xb_t = _es.enter_context(nc.sbuf_tensor("xb_t", [_P, _FEAT], bf16))
            diff_t = _es.enter_context(nc.sbuf_tensor("diff_t", [_P, _FEAT], bf16))
        lab_sem = _es.enter_context(nc.semaphore("lab_sem"))
        lab_b_sem = _es.enter_context(nc.semaphore("lab_b_sem"))
        x_sem = _es.enter_context(nc.semaphore("x_sem"))
        n_sem = _es.enter_context(nc.semaphore("n_sem"))
        c_sem = _es.enter_context(nc.semaphore("c_sem"))
        dve_sem = _es.enter_context(nc.semaphore("dve_sem"))
        m_sem = _es.enter_context(nc.semaphore("m_sem"))
        o_sem = _es.enter_context(nc.semaphore("o_sem"))
        # labels sprays lead each HWDGE ring; x row-halves queue behind them
        # on the same rings so the label descriptors drain first
        nc.sync.dma_start(labels_t.ap()[0:_H, :], labels_d[0:_H, :]).then_inc(
            lab_sem, 16
        )
        nc.scalar.dma_start(labels_t.ap()[_H:_P, :], labels_d[_H:_ROWS, :]).then_inc(
            lab_b_sem, 16
        )
        nc.sync.dma_start(x_t.ap()[0:_H, :], x_d[0:_H, :]).then_inc(x_sem, 16)
        nc.scalar.dma_start(x_t.ap()[_H:_P, :], x_d[_H:_ROWS, :]).then_inc(x_sem, 16)
        nc.vector.memset(ones_t.ap(), 1.0)

        if fused:
            # prefill d = -x (cast); the gather's CCE add turns it into c - x
            nc.vector.wait_ge(x_sem, 32)
            nc.vector.tensor_scalar_mul(d_t.ap(), x_t.ap(), -1.0).then_inc(n_sem, 1)
            nc.gpsimd.wait_ge(n_sem, 1)
        else:
            # ACT casts x to bf16 while the gather is in flight
            nc.scalar.wait_ge(x_sem, 32)
            nc.scalar.copy(xb_t.ap(), x_t.ap()).then_inc(n_sem, 1)

        nc.gpsimd.wait_ge(lab_sem, 16)
        nc.gpsimd.wait_ge(lab_b_sem, 16)
        nc.gpsimd.indirect_dma_start(
            out=d_t.ap(),
            out_offset=None,
            in_=centers_d,
            in_offset=bass.IndirectOffsetOnAxis(ap=labels_t.ap()[:, :1], axis=0),
            compute_op=(
                mybir.AluOpType.add if fused else mybir.AluOpType.bypass
            ),
        ).then_inc(c_sem, 16)

        nc.vector.wait_ge(c_sem, 16)
        if not fused:
            nc.vector.wait_ge(n_sem, 1)
            nc.vector.tensor_tensor(
                out=diff_t.ap(), in0=d_t.ap(), in1=xb_t.ap(),
                op=mybir.AluOpType.subtract,
            ).then_inc(dve_sem, 1)
            nc.vector.wait_ge(dve_sem, 1)
            sq_in = diff_t
            base = 1
        else:
            sq_in = d_t
            base = 0
        nc.vector.scalar_tensor_tensor(
            out=sq_t.ap(), in0=sq_in.ap(), scalar=1.0, in1=sq_in.ap(),
            op0=mybir.AluOpType.mult, op1=mybir.AluOpType.mult,
            accum_out=s1_t.ap(),
        ).then_inc(dve_sem, 1)

        nc.tensor.wait_ge(dve_sem, base + 1)
        nc.tensor.matmul(
            acc_t.ap(), lhsT=s1_t.ap(), rhs=ones_t.ap(), start=True, stop=True
        ).then_inc(m_sem, 1)

        nc.vector.wait_ge(m_sem, 1)
        nc.vector.tensor_copy(out=res_t.ap(), in_=acc_t.ap()).then_inc(dve_sem, 1)

        nc.sync.wait_ge(dve_sem, base + 2)
        nc.sync.dma_start(out_d, res_t.ap()).then_inc(o_sem, 16)

    nc.compile()
    return nc


def _run(x, labels, centers, trace=False, mode=MODE, **_ignored):
    from concourse.bass_utils import run_bass_kernel_spmd

    key = ("nc", mode)
    if key not in _state:
        _state[key] = _build_nc_raw(mode=mode)
    nc = _state[key]

    x = np.ascontiguousarray(np.asarray(x, dtype=np.float32)).reshape(
        _NCORES, _ROWS, _FEAT
    )
    lab = (
        np.ascontiguousarray(np.asarray(labels))
        .astype(np.int32)
        .reshape(_NCORES, _ROWS, 1)
    )
    cen = np.ascontiguousarray(np.asarray(centers, dtype=np.float32))
    in_maps = [{"x": x[i], "labels": lab[i], "centers": cen} for i in range(_NCORES)]
    res = run_bass_kernel_spmd(nc, in_maps, core_ids=list(range(_NCORES)), trace=trace)
    total = 0.0
    for r in res.results:
        total += float(r["out"][0, 0])
    loss = total / _BATCH + (_NCLASSES - 1) * 1e-12
    return np.float32(loss), res


def kernel(x, labels, centers):
    loss, _ = _run(x, labels, centers, trace=False)
    return loss


# revision 13
# speedup vs baseline: 1.1379x; 1.1379x over previous
"""Raw-Bacc (no TileContext) CenterLoss kernel.

loss = mean_b ||x_b - centers[labels_b]||^2  (+ tiny clip-floor constant)

Per core (128 batch rows), shipped mode "diff_bf16_sr" (single-ring):
  SP queue:  labels [128,1] spray, then x [128,512] BEHIND it on the SAME
             ring.  Ring-FIFO guarantees every SDMA engine drains its label
             descriptors (and the labels DMA's semaphore descriptor) before
             any 2KB x descriptor; the ACT ring carries no DMA, so no
             cross-ring packet arbitration can delay the labels semaphore.
             (With labels/x split across both HWDGE rings, the trace showed
             the labels sem's 16 engine increments trickling over ~0.65us
             behind the other ring's x packets; single-ring cuts the
             gather's start by ~1us and collapses run variance to ~0.1us.)
  ACT queue: casts x -> bf16 while the gather is in flight
  Pool:      single indirect DMA gathers centers[labels] rows, cast to bf16
             on the fly (enables 2x-rate DVE math; HBM reads are unchanged)
  DVE:       diff = c - x (bf16, 2x rate); sq = diff*diff, f32 row-accum -> s1
  PE:        ones-matmul reduces the 128 partition values to one scalar
  DVE:       PSUM -> SBUF copy;  SP: DMA scalar out
Host sums the 8 per-core partials (the all-reduce) and divides by B.

The clip in the reference only matters at ~1e-11 relative (distances are
~1024, the 1e-12 floor applies to the zeroed off-label entries, added back
as a host-side constant), so it is dropped on-device.

Modes kept for reference: "fused_bf16"/"fused_f32" pre-fill the gather
destination with -x and use the DMA CCE compute_op=add so the drain itself
produces c - x.  Measured SLOWER on HW (+~1.5us: desc-gen with compute_op
is ~0.6us longer and the per-descriptor RMW slows the drain more than the
saved DVE op).  Split gathers and non-[128,1] offset APs crash the HW
indirect-DMA ucode; offsets must live in SBUF, one per output partition.
"""

import numpy as np

_BATCH = 1024
_FEAT = 512
_NCLASSES = 10000
_NCORES = 8
_ROWS = _BATCH // _NCORES  # 128
_P = 128
_H = _ROWS // 2  # 64

MODE = "diff_bf16_sr"

_state = {}


def _build_nc_raw(mode=MODE):
    import concourse.bass as bass
    import concourse.mybir as mybir
    from concourse import bacc

    f32 = mybir.dt.float32
    bf16 = mybir.dt.bfloat16
    i32 = mybir.dt.int32
    fused = mode in ("fused_bf16", "fused_f32")
    single_ring = mode == "diff_bf16_sr"
    gdt = f32 if mode == "fused_f32" else bf16

    nc = bacc.Bacc("TRN2", target_bir_lowering=False, debug=False)
    x_d = nc.dram_tensor("x", [_ROWS, _FEAT], f32, kind="ExternalInput").ap()
    labels_d = nc.dram_tensor("labels", [_ROWS, 1], i32, kind="ExternalInput").ap()
    centers_d = nc.dram_tensor(
        "centers", [_NCLASSES, _FEAT], f32, kind="ExternalInput"
    ).ap()
    out_d = nc.dram_tensor("out", [1, 1], f32, kind="ExternalOutput").ap()

    from contextlib import ExitStack

    with ExitStack() as _es:
        labels_t = _es.enter_context(nc.sbuf_tensor("labels_t", [_ROWS, 1], i32))
        x_t = _es.enter_context(nc.sbuf_tensor("x_t", [_P, _FEAT], f32))
        d_t = _es.enter_context(nc.sbuf_tensor("d_t", [_P, _FEAT], gdt))
        sq_t = _es.enter_context(nc.sbuf_tensor("sq_t", [_P, _FEAT], gdt))
        s1_t = _es.enter_context(nc.sbuf_tensor("s1_t", [_P, 1], f32))
        ones_t = _es.enter_context(nc.sbuf_tensor("ones_t", [_P, 1], f32))
        res_t = _es.enter_context(nc.sbuf_tensor("res_t", [1, 1], f32))
        acc_t = _es.enter_context(nc.psum_tensor("acc_t", [1, 1], f32))
        if not fused:
            xb_t = _es.enter_context(nc.sbuf_tensor("xb_t", [_P, _FEAT], bf16))
            diff_t = _es.enter_context(nc.sbuf_tensor("diff_t", [_P, _FEAT], bf16))
        lab_sem = _es.enter_context(nc.semaphore("lab_sem"))
        lab_b_sem = _es.enter_context(nc.semaphore("lab_b_sem"))
        x_sem = _es.enter_context(nc.semaphore("x_sem"))
        n_sem = _es.enter_context(nc.semaphore("n_sem"))
        c_sem = _es.enter_context(nc.semaphore("c_sem"))
        dve_sem = _es.enter_context(nc.semaphore("dve_sem"))
        m_sem = _es.enter_context(nc.semaphore("m_sem"))
        o_sem = _es.enter_context(nc.semaphore("o_sem"))
        if single_ring:
            # everything on the SP ring, labels first: the scalar ring stays
            # empty so no 2KB x descriptors interleave with the labels spray
            # on the shared SDMA engines (trace showed the labels sem's 16
            # increments trickling over ~0.65us behind cross-ring x packets)
            nc.sync.dma_start(labels_t.ap(), labels_d).then_inc(lab_sem, 16)
            nc.sync.dma_start(x_t.ap(), x_d).then_inc(x_sem, 16)
            x_target = 16
        else:
            # labels sprays lead each HWDGE ring; x row-halves queue behind
            # them on the same rings so the label descriptors drain first
            nc.sync.dma_start(labels_t.ap()[0:_H, :], labels_d[0:_H, :]).then_inc(
                lab_sem, 16
            )
            nc.scalar.dma_start(
                labels_t.ap()[_H:_P, :], labels_d[_H:_ROWS, :]
            ).then_inc(lab_b_sem, 16)
            nc.sync.dma_start(x_t.ap()[0:_H, :], x_d[0:_H, :]).then_inc(x_sem, 16)
            nc.scalar.dma_start(x_t.ap()[_H:_P, :], x_d[_H:_ROWS, :]).then_inc(
                x_sem, 16
            )
            x_target = 32
        nc.vector.memset(ones_t.ap(), 1.0)

        if fused:
            # prefill d = -x (cast); the gather's CCE add turns it into c - x
            nc.vector.wait_ge(x_sem, x_target)
            nc.vector.tensor_scalar_mul(d_t.ap(), x_t.ap(), -1.0).then_inc(n_sem, 1)
            nc.gpsimd.wait_ge(n_sem, 1)
        else:
            # ACT casts x to bf16 while the gather is in flight
            nc.scalar.wait_ge(x_sem, x_target)
            nc.scalar.copy(xb_t.ap(), x_t.ap()).then_inc(n_sem, 1)

        nc.gpsimd.wait_ge(lab_sem, 16)
        if not single_ring:
            nc.gpsimd.wait_ge(lab_b_sem, 16)
        nc.gpsimd.indirect_dma_start(
            out=d_t.ap(),
            out_offset=None,
            in_=centers_d,
            in_offset=bass.IndirectOffsetOnAxis(ap=labels_t.ap()[:, :1], axis=0),
            compute_op=(
                mybir.AluOpType.add if fused else mybir.AluOpType.bypass
            ),
        ).then_inc(c_sem, 16)

        nc.vector.wait_ge(c_sem, 16)
        if not fused:
            nc.vector.wait_ge(n_sem, 1)
            nc.vector.tensor_tensor(
                out=diff_t.ap(), in0=d_t.ap(), in1=xb_t.ap(),
                op=mybir.AluOpType.subtract,
            ).then_inc(dve_sem, 1)
            nc.vector.wait_ge(dve_sem, 1)
            sq_in = diff_t
            base = 1
        else:
            sq_in = d_t
            base = 0
        nc.vector.scalar_tensor_tensor(
            out=sq_t.ap(), in0=sq_in.ap(), scalar=1.0, in1=sq_in.ap(),
            op0=mybir.AluOpType.mult, op1=mybir.AluOpType.mult,
            accum_out=s1_t.ap(),
        ).then_inc(dve_sem, 1)

        nc.tensor.wait_ge(dve_sem, base + 1)
        nc.tensor.matmul(
            acc_t.ap(), lhsT=s1_t.ap(), rhs=ones_t.ap(), start=True, stop=True
        ).then_inc(m_sem, 1)

        nc.vector.wait_ge(m_sem, 1)
        nc.vector.tensor_copy(out=res_t.ap(), in_=acc_t.ap()).then_inc(dve_sem, 1)

        nc.sync.wait_ge(dve_sem, base + 2)
        nc.sync.dma_start(out_d, res_t.ap()).then_inc(o_sem, 16)

    nc.compile()
    return nc


def _run(x, labels, centers, trace=False, mode=MODE, **_ignored):
    from concourse.bass_utils import run_bass_kernel_spmd

    key = ("nc", mode)
    if key not in _state:
        _state[key] = _build_nc_raw(mode=mode)
    nc = _state[key]

    x = np.ascontiguousarray(np.asarray(x, dtype=np.float32)).reshape(
        _NCORES, _ROWS, _FEAT
    )
    lab = (
        np.ascontiguousarray(np.asarray(labels))
        .astype(np.int32)
        .reshape(_NCORES, _ROWS, 1)
    )
    cen = np.ascontiguousarray(np.asarray(centers, dtype=np.float32))
    in_maps = [{"x": x[i], "labels": lab[i], "centers": cen} for i in range(_NCORES)]
    res = run_bass_kernel_spmd(nc, in_maps, core_ids=list(range(_NCORES)), trace=trace)
    total = 0.0
    for r in res.results:
        total += float(r["out"][0, 0])
    loss = total / _BATCH + (_NCLASSES - 1) * 1e-12
    return np.float32(loss), res


def kernel(x, labels, centers):
    loss, _ = _run(x, labels, centers, trace=False)
    return loss


# revision 16
# speedup vs baseline: 1.1659x; 1.0246x over previous
"""Raw-Bacc (no TileContext) CenterLoss kernel.

loss = mean_b ||x_b - centers[labels_b]||^2  (+ tiny clip-floor constant)

Per core (128 batch rows), shipped mode "diff_bf16_sr" (single-ring):
  SP queue:  labels [128,1] spray, then x [128,512] BEHIND it on the SAME
             ring.  Ring-FIFO guarantees every SDMA engine drains its label
             descriptors (and the labels DMA's semaphore descriptor) before
             any 2KB x descriptor; the ACT ring carries no DMA, so no
             cross-ring packet arbitration can delay the labels semaphore.
             (With labels/x split across both HWDGE rings, the trace showed
             the labels sem's 16 engine increments trickling over ~0.65us
             behind the other ring's x packets; single-ring cuts the
             gather's start by ~1us and collapses run variance to ~0.1us.)
  ACT queue: casts x -> bf16 while the gather is in flight
  Pool:      single indirect DMA gathers centers[labels] rows, cast to bf16
             on the fly (enables 2x-rate DVE math; HBM reads are unchanged)
  DVE:       diff = c - x (bf16, 2x rate); sq = diff*diff, f32 row-accum -> s1
  PE:        ones-matmul reduces the 128 partition values to one scalar
  DVE:       PSUM -> SBUF copy;  SP: DMA scalar out
Host sums the 8 per-core partials (the all-reduce) and divides by B.

The clip in the reference only matters at ~1e-11 relative (distances are
~1024, the 1e-12 floor applies to the zeroed off-label entries, added back
as a host-side constant), so it is dropped on-device.

Modes kept for reference: "fused_bf16"/"fused_f32" pre-fill the gather
destination with -x and use the DMA CCE compute_op=add so the drain itself
produces c - x.  Measured SLOWER on HW (+~1.5us: desc-gen with compute_op
is ~0.6us longer and the per-descriptor RMW slows the drain more than the
saved DVE op).  Split gathers and non-[128,1] offset APs crash the HW
indirect-DMA ucode; offsets must live in SBUF, one per output partition.
"""

import numpy as np

_BATCH = 1024
_FEAT = 512
_NCLASSES = 10000
_NCORES = 8
_ROWS = _BATCH // _NCORES  # 128
_P = 128
_H = _ROWS // 2  # 64

MODE = "diff_bf16_sr"

_state = {}


def _build_nc_raw(mode=MODE):
    import concourse.bass as bass
    import concourse.mybir as mybir
    from concourse import bacc

    f32 = mybir.dt.float32
    bf16 = mybir.dt.bfloat16
    i32 = mybir.dt.int32
    fused = mode in ("fused_bf16", "fused_f32")
    single_ring = mode in ("diff_bf16_sr", "diff_bf16_sr2")
    sr2 = mode == "diff_bf16_sr2"
    gdt = f32 if mode == "fused_f32" else bf16

    nc = bacc.Bacc("TRN2", target_bir_lowering=False, debug=False)
    x_d = nc.dram_tensor("x", [_ROWS, _FEAT], f32, kind="ExternalInput").ap()
    labels_d = nc.dram_tensor("labels", [_ROWS, 1], i32, kind="ExternalInput").ap()
    centers_d = nc.dram_tensor(
        "centers", [_NCLASSES, _FEAT], f32, kind="ExternalInput"
    ).ap()
    out_d = nc.dram_tensor("out", [1, 1], f32, kind="ExternalOutput").ap()

    from contextlib import ExitStack

    with ExitStack() as _es:
        labels_t = _es.enter_context(nc.sbuf_tensor("labels_t", [_ROWS, 1], i32))
        x_t = _es.enter_context(nc.sbuf_tensor("x_t", [_P, _FEAT], f32))
        d_t = _es.enter_context(nc.sbuf_tensor("d_t", [_P, _FEAT], gdt))
        sq_t = _es.enter_context(nc.sbuf_tensor("sq_t", [_P, _FEAT], gdt))
        s1_t = _es.enter_context(nc.sbuf_tensor("s1_t", [_P, 1], f32))
        ones_t = _es.enter_context(nc.sbuf_tensor("ones_t", [_P, 1], f32))
        res_t = _es.enter_context(nc.sbuf_tensor("res_t", [1, 1], f32))
        acc_t = _es.enter_context(nc.psum_tensor("acc_t", [1, 1], f32))
        if not fused:
            xb_t = _es.enter_context(nc.sbuf_tensor("xb_t", [_P, _FEAT], bf16))
            diff_t = _es.enter_context(nc.sbuf_tensor("diff_t", [_P, _FEAT], bf16))
        lab_sem = _es.enter_context(nc.semaphore("lab_sem"))
        lab_b_sem = _es.enter_context(nc.semaphore("lab_b_sem"))
        x_sem = _es.enter_context(nc.semaphore("x_sem"))
        n_sem = _es.enter_context(nc.semaphore("n_sem"))
        c_sem = _es.enter_context(nc.semaphore("c_sem"))
        dve_sem = _es.enter_context(nc.semaphore("dve_sem"))
        m_sem = _es.enter_context(nc.semaphore("m_sem"))
        o_sem = _es.enter_context(nc.semaphore("o_sem"))
        if single_ring:
            # everything on ONE HWDGE ring, labels first: the other ring
            # stays empty so no 2KB x descriptors interleave with the labels
            # spray on the shared SDMA engines (trace showed the labels sem's
            # 16 increments trickling over ~0.65us behind cross-ring packets)
            ldq = nc.scalar if sr2 else nc.sync
            ldq.dma_start(labels_t.ap(), labels_d).then_inc(lab_sem, 16)
            ldq.dma_start(x_t.ap(), x_d).then_inc(x_sem, 16)
            x_target = 16
        else:
            # labels sprays lead each HWDGE ring; x row-halves queue behind
            # them on the same rings so the label descriptors drain first
            nc.sync.dma_start(labels_t.ap()[0:_H, :], labels_d[0:_H, :]).then_inc(
                lab_sem, 16
            )
            nc.scalar.dma_start(
                labels_t.ap()[_H:_P, :], labels_d[_H:_ROWS, :]
            ).then_inc(lab_b_sem, 16)
            nc.sync.dma_start(x_t.ap()[0:_H, :], x_d[0:_H, :]).then_inc(x_sem, 16)
            nc.scalar.dma_start(x_t.ap()[_H:_P, :], x_d[_H:_ROWS, :]).then_inc(
                x_sem, 16
            )
            x_target = 32
        nc.vector.memset(ones_t.ap(), 1.0)

        if fused:
            # prefill d = -x (cast); the gather's CCE add turns it into c - x
            nc.vector.wait_ge(x_sem, x_target)
            nc.vector.tensor_scalar_mul(d_t.ap(), x_t.ap(), -1.0).then_inc(n_sem, 1)
            nc.gpsimd.wait_ge(n_sem, 1)
        elif sr2:
            # DVE casts x to bf16 while the gather is in flight (scalar
            # queue carries the loads; DVE is idle in this window anyway)
            nc.vector.wait_ge(x_sem, x_target)
            nc.vector.tensor_copy(out=xb_t.ap(), in_=x_t.ap()).then_inc(n_sem, 1)
        else:
            # ACT casts x to bf16 while the gather is in flight
            nc.scalar.wait_ge(x_sem, x_target)
            nc.scalar.copy(xb_t.ap(), x_t.ap()).then_inc(n_sem, 1)

        nc.gpsimd.wait_ge(lab_sem, 16)
        if not single_ring:
            nc.gpsimd.wait_ge(lab_b_sem, 16)
        nc.gpsimd.indirect_dma_start(
            out=d_t.ap(),
            out_offset=None,
            in_=centers_d,
            in_offset=bass.IndirectOffsetOnAxis(ap=labels_t.ap()[:, :1], axis=0),
            compute_op=(
                mybir.AluOpType.add if fused else mybir.AluOpType.bypass
            ),
        ).then_inc(c_sem, 16)

        nc.vector.wait_ge(c_sem, 16)
        if not fused:
            nc.vector.wait_ge(n_sem, 1)
            nc.vector.tensor_tensor(
                out=diff_t.ap(), in0=d_t.ap(), in1=xb_t.ap(),
                op=mybir.AluOpType.subtract,
            ).then_inc(dve_sem, 1)
            nc.vector.wait_ge(dve_sem, 1)
            sq_in = diff_t
            base = 1
        else:
            sq_in = d_t
            base = 0
        nc.vector.scalar_tensor_tensor(
            out=sq_t.ap(), in0=sq_in.ap(), scalar=1.0, in1=sq_in.ap(),
            op0=mybir.AluOpType.mult, op1=mybir.AluOpType.mult,
            accum_out=s1_t.ap(),
        ).then_inc(dve_sem, 1)

        nc.tensor.wait_ge(dve_sem, base + 1)
        nc.tensor.matmul(
            acc_t.ap(), lhsT=s1_t.ap(), rhs=ones_t.ap(), start=True, stop=True
        ).then_inc(m_sem, 1)

        nc.vector.wait_ge(m_sem, 1)
        nc.vector.tensor_copy(out=res_t.ap(), in_=acc_t.ap()).then_inc(dve_sem, 1)

        nc.sync.wait_ge(dve_sem, base + 2)
        nc.sync.dma_start(out_d, res_t.ap()).then_inc(o_sem, 16)

    nc.compile()
    return nc


def _run(x, labels, centers, trace=False, mode=MODE, **_ignored):
    from concourse.bass_utils import run_bass_kernel_spmd

    key = ("nc", mode)
    if key not in _state:
        _state[key] = _build_nc_raw(mode=mode)
    nc = _state[key]

    x = np.ascontiguousarray(np.asarray(x, dtype=np.float32)).reshape(
        _NCORES, _ROWS, _FEAT
    )
    lab = (
        np.ascontiguousarray(np.asarray(labels))
        .astype(np.int32)
        .reshape(_NCORES, _ROWS, 1)
    )
    cen = np.ascontiguousarray(np.asarray(centers, dtype=np.float32))
    in_maps = [{"x": x[i], "labels": lab[i], "centers": cen} for i in range(_NCORES)]
    res = run_bass_kernel_spmd(nc, in_maps, core_ids=list(range(_NCORES)), trace=trace)
    total = 0.0
    for r in res.results:
        total += float(r["out"][0, 0])
    loss = total / _BATCH + (_NCLASSES - 1) * 1e-12
    return np.float32(loss), res


def kernel(x, labels, centers):
    loss, _ = _run(x, labels, centers, trace=False)
    return loss
